# revision 19
# baseline (speedup 1.0000x reference)
"""Trainium2 Bass kernel for nn_DecoderLayer (gnn_message_passing).

Sharding: flatten B*N = 4096 nodes, 512 nodes per core across 8 cores.
Device layout is feature-on-partition (transposed); h_E is pre-transposed and
cast to bf16 on host so the big stream DMAs straight into bf16 matmuls.

Math per node n, neighbor k (reference):
  h_EV = [h_V[n], h_E[n,k]]                                (128+384)
  h1 = gelu(h_EV @ W1.T + b1); h2 = gelu(h1 @ W2.T + b2)
  msg = h2 @ W3.T + b3
  dh  = sum_k mask_attend[n,k] * msg / 30
  h   = LN1(h_V + dh)
  dh2 = gelu(h @ Win.T + bin) @ Wout.T + bout
  out = mask_V[n] * LN2(h + dh2)

Structure (from trace analysis):
  - the scalar (ACT) engine is the hard floor: two full gelu passes over
    B*N*K rows at 1 elem/lane/cycle.  Everything else hides behind it.
  - stream path all bf16 (fast PE mode + FWL; fp32 matmuls run a slow mode
    and draw the HAM power throttle).
  - W1 split: hv1 = W1V^T h_V + b1 computed once per node, broadcast across
    K into PSUM via a bf16 identity matmul with a stride-0 moving AP.
  - mask_attend folded into the data host-side (poisoned h_E rows make the
    masked gelu exactly 0; the constant gelu(b2) leak is corrected through
    the W3 constants).  No mask work on the device.
  - K-sum before W3 (linear commutes): m2[n] = sum_k h2.
  - the per-node tail (W3, LN1, FFN, LN2, store) is cut into 6 tasks per
    128-node block and drip-fed into the stream one task per matmul pair,
    so it overlaps the ACT-bound stream instead of serializing at the end.
  - LayerNorm entirely on the vector engine (rsqrt via bitcast-Newton):
    the ACT table never leaves the gelu set, so no 2.7us table swaps.
"""

from contextlib import ExitStack

import numpy as np

import concourse.bacc as bacc
import concourse.tile as tile
from concourse import mybir
from concourse.bass_utils import run_bass_kernel_spmd

F32 = mybir.dt.float32
BF16 = mybir.dt.bfloat16
I32 = mybir.dt.int32
AF = mybir.ActivationFunctionType
ALU = mybir.AluOpType
AX = mybir.AxisListType

H = 128
NIN = 384
FF = 4 * H
NCHUNK = NIN // 128  # 3
FCHUNK = FF // 128   # 4
K = 48
SCALE = 30.0
EPS = 1e-5
NCORES = 8
POISON = 50.0

TT = 384                 # rows per matmul tile (8 nodes * 48)
NPT = TT // K            # 8 nodes per tile
PAIR = 2 * TT            # rows per activation/DVE pass
DMA_GROUP = 4            # tiles per DMA load
G_ROWS = TT * DMA_GROUP  # 1536 rows (1.18 MB bf16) per load

GELU = AF.Gelu

# packed f32 const layout (columns)
_PK32 = {"b1": (0, 1), "b2": (1, 1), "epsv": (2, 1), "dhbias": (3, 1),
         "magic4": (4, 4), "bout": (8, 1), "binp": (9, 4),
         "b3rep": (16, 128), "g1rep": (144, 128), "b1rep": (272, 128),
         "g2rep": (400, 128), "b2rep": (528, 128), "identf": (656, 128)}
PK32_HOT = 16
PK32_COLS = 784
# packed bf16 const layout
_PKB = {"w1vt": (0, 128), "identb": (128, 128), "w1et": (256, NCHUNK * 128),
        "w2t": (640, 128), "w3t": (768, 128), "wint": (896, FF),
        "woutt": (1408, FCHUNK * 128)}
PKB_HOT = 768
PKB_COLS = 1920


def _emit(nc, io, npc):
    rows = npc * K
    ngrp = rows // G_ROWS
    nblk = npc // 128
    assert rows % G_ROWS == 0 and npc % 128 == 0

    with tile.TileContext(nc) as tc, ExitStack() as ctx:
        cpool = ctx.enter_context(tc.tile_pool(name="const", bufs=1))
        small = ctx.enter_context(tc.tile_pool(name="small", bufs=8))
        hpool = ctx.enter_context(tc.tile_pool(name="he", bufs=4))
        wpool = ctx.enter_context(tc.tile_pool(name="work", bufs=6))
        tpool = ctx.enter_context(tc.tile_pool(name="tail", bufs=2))

        # ---- packed constants (few big DMAs) ----
        # warm the Gelu LUT first; input from a memset tile so the table
        # load doesn't wait on any DMA
        wz = small.tile([128, 1], F32, tag="wz")
        nc.gpsimd.memset(wz[:], 0.0)
        warm = small.tile([128, 1], F32, tag="warm")
        nc.scalar.activation(warm[:], wz[:], GELU)

        # hot weights ride the fast sync DMA ring (~3.5x the gpsimd
        # ring's effective bandwidth) so the stream starts sooner
        pkb = cpool.tile([128, PKB_COLS], BF16, tag="pkb")
        nc.sync.dma_start(pkb[:, 0:PKB_HOT], io["pkb"][:, 0:PKB_HOT])
        hv_tb = cpool.tile([128, npc], BF16, tag="hv_tb")
        nc.sync.dma_start(hv_tb[:], io["hv_t"][:])
        pk32 = cpool.tile([128, PK32_COLS], F32, tag="pk32")
        nc.sync.dma_start(pk32[:, 0:PK32_HOT], io["pk32"][:, 0:PK32_HOT])
        nc.gpsimd.dma_start(pkb[:, PKB_HOT:], io["pkb"][:, PKB_HOT:])
        nc.gpsimd.dma_start(pk32[:, PK32_HOT:], io["pk32"][:, PK32_HOT:])

        def c32(name):
            o, w = _PK32[name]
            return pk32[:, o:o + w]

        def cb(name):
            o, w = _PKB[name]
            return pkb[:, o:o + w]

        hv_nat = cpool.tile([128, nblk * 128], F32, tag="hv_nat")
        nc.gpsimd.dma_start(
            hv_nat[:], io["hv_nat"][:].rearrange("(b p) f -> p b f", p=128))
        s_mask = cpool.tile([128, nblk], F32, tag="s_mask")
        nc.gpsimd.dma_start(s_mask[:], io["s_mask"][:])
        maskv = cpool.tile([128, nblk], F32, tag="maskv")
        nc.gpsimd.dma_start(maskv[:], io["maskv_nat"][:])

        h_et = io["h_et"][:]            # [NCHUNK, 128, rows] bf16

        def load_group(g):
            he = hpool.tile([128, NCHUNK * G_ROWS], BF16, tag="he")
            r0 = g * G_ROWS
            # src (p, c, r) enumeration to match dest free layout (c, r)
            nc.sync.dma_start(
                he[:], h_et[:, :, r0:r0 + G_ROWS].transpose([1, 0, 2]))
            return he

        he_q = [load_group(0), load_group(1)]  # prefetch before barrier

        m2 = cpool.tile([128, npc], BF16, tag="m2")

        # hv1 = W1V^T @ h_V + b1, computed once, rounded to bf16
        hv1b = cpool.tile([128, npc], BF16, tag="hv1b")
        with tc.tile_pool(name="pp0", bufs=1, space="PSUM") as pp0:
            ps_hv = pp0.tile([128, npc], F32, tag="pp0")
            nc.tensor.matmul(ps_hv[:], cb("w1vt"), hv_tb[:],
                             start=True, stop=True)
            nc.scalar.activation(hv1b[:], ps_hv[:], AF.Identity,
                                 bias=c32("b1"))

        # per-node constant terms, precomputed off the critical path:
        # hvterm = h_V + s_mask*b3c (enters LN1 residual);
        # mvb2 = mask_V * ln2_b (enters the masked output)
        hvterm = cpool.tile([128, nblk * 128], F32, tag="hvterm")
        nc.vector.tensor_tensor(
            hvterm[:].rearrange("p (b f) -> p b f", f=128),
            s_mask[:].unsqueeze(2).broadcast_to([128, nblk, 128]),
            c32("b3rep").unsqueeze(1).broadcast_to([128, nblk, 128]),
            ALU.mult)
        nc.vector.tensor_tensor(hvterm[:], hvterm[:], hv_nat[:], ALU.add)
        mvb2 = cpool.tile([128, nblk * 128], F32, tag="mvb2")
        nc.vector.tensor_tensor(
            mvb2[:].rearrange("p (b f) -> p b f", f=128),
            maskv[:].unsqueeze(2).broadcast_to([128, nblk, 128]),
            c32("b2rep").unsqueeze(1).broadcast_to([128, nblk, 128]),
            ALU.mult)

        def rsqrt_blk(v1, pfx):
            # rstd = 1/sqrt(v1) on [128,1] via bitcast + 2 Newton steps
            yi = small.tile([128, 1], I32, tag=pfx + "yi")
            nc.vector.tensor_scalar(yi[:], v1.bitcast(I32), 1, None,
                                    ALU.arith_shift_right)
            nc.vector.tensor_tensor(yi[:], c32("magic4")[:, 0:1].bitcast(I32),
                                    yi[:], ALU.subtract)
            y = yi[:].bitcast(F32)
            s = small.tile([128, 1], F32, tag=pfx + "s")
            for _ in range(1):  # y *= 1.5 - 0.5*v*y*y
                nc.vector.tensor_tensor(s[:], y, y, ALU.mult)
                nc.vector.tensor_tensor(s[:], s[:], v1, ALU.mult)
                nc.vector.tensor_scalar(s[:], s[:], -0.5, 1.5,
                                        ALU.mult, ALU.add)
                nc.vector.tensor_tensor(y, y, s[:], ALU.mult)
            return yi[:].bitcast(F32)

        def normalize_blk(x, pfx):
            # x: [128,128] fp32, normalized in place to (x-mu)/std
            mu = small.tile([128, 1], F32, tag=pfx + "mu")
            nc.vector.tensor_reduce(mu[:], x, AX.X, ALU.add)
            nc.vector.tensor_scalar_mul(mu[:], mu[:], 1.0 / 128.0)
            nc.vector.tensor_scalar_sub(x, x, mu[:, 0:1])
            sq = tpool.tile([128, 128], F32, tag="lnsq")
            nc.vector.tensor_tensor(sq[:], x, x, ALU.mult)
            v1 = small.tile([128, 1], F32, tag=pfx + "v1")
            nc.vector.tensor_reduce(v1[:], sq[:], AX.X, ALU.add)
            nc.vector.tensor_scalar(v1[:], v1[:], 1.0 / 128.0, EPS,
                                    ALU.mult, ALU.add)
            rstd = rsqrt_blk(v1[:], pfx)
            nc.vector.tensor_scalar_mul(x, x, rstd[:, 0:1])

        # ---- per-block tail tasks (drip-fed into the stream) ----
        blk = [{} for _ in range(nblk)]
        tail_tasks = []

        def make_tasks(j, tailp):
            js = slice(j * 128, (j + 1) * 128)

            def t1():
                ps = tailp.tile([128, 512], F32, tag="tps")
                nc.tensor.matmul(ps[:, 0:128], cb("w3t"), m2[:, js],
                                 start=True, stop=True)
                d = tpool.tile([128, 128], F32, tag="dht")
                nc.vector.tensor_scalar_add(d[:], ps[:, 0:128],
                                            c32("dhbias")[:, 0:1])
                blk[j]["dht"] = d

            def t2():
                pn = tailp.tile([128, 512], F32, tag="tps")
                nc.tensor.transpose(pn[:, 0:128], blk[j]["dht"][:],
                                    c32("identf"))
                x = tpool.tile([128, 128], F32, tag="x1")
                nc.vector.tensor_tensor(x[:], pn[:, 0:128], hvterm[:, js],
                                        ALU.add)
                normalize_blk(x[:], "ln1_%d" % j)
                blk[j]["xh"] = x

            def t3():
                # FFN path: ln1 affine is folded into wint/binp on host, so
                # the transpose consumes the bare normalized x-hat
                pt = tailp.tile([128, 512], F32, tag="tps")
                nc.tensor.transpose(pt[:, 0:128], blk[j]["xh"][:],
                                    c32("identf"))
                ht = tpool.tile([128, 128], BF16, tag="ht")
                nc.vector.tensor_scalar_add(ht[:], pt[:, 0:128], 0.0)
                blk[j]["ht"] = ht
                # residual-path h = xh*g1 + b1, off the FFN critical chain
                h = tpool.tile([128, 128], F32, tag="hj")
                nc.vector.tensor_tensor(h[:], blk[j]["xh"][:], c32("g1rep"),
                                        ALU.mult)
                nc.vector.tensor_tensor(h[:], h[:], c32("b1rep"), ALU.add)
                blk[j]["h"] = h

            def t4():
                pf = tailp.tile([128, 512], F32, tag="tps")
                for jo in range(FCHUNK):
                    nc.tensor.matmul(
                        pf[:, jo * 128:(jo + 1) * 128],
                        cb("wint")[:, jo * 128:(jo + 1) * 128],
                        blk[j]["ht"][:], start=True, stop=True)
                fz = tpool.tile([128, 512], BF16, tag="fz")
                nc.vector.tensor_tensor(
                    fz[:].rearrange("p (c f) -> p c f", f=128),
                    pf[:].rearrange("p (c f) -> p c f", f=128),
                    c32("binp").unsqueeze(2).broadcast_to([128, FCHUNK, 128]),
                    ALU.add)
                fb = tpool.tile([128, 512], BF16, tag="fb")
                nc.scalar.activation(fb[:], fz[:], GELU)
                blk[j]["fb"] = fb

            def t5():
                ps2_ = tailp.tile([128, 512], F32, tag="tps")
                for jf in range(FCHUNK):
                    nc.tensor.matmul(
                        ps2_[:, 0:128],
                        cb("woutt")[:, jf * 128:(jf + 1) * 128],
                        blk[j]["fb"][:, jf * 128:(jf + 1) * 128],
                        start=(jf == 0), stop=(jf == FCHUNK - 1))
                d2 = tpool.tile([128, 128], F32, tag="dh2")
                nc.vector.tensor_scalar_add(d2[:], ps2_[:, 0:128],
                                            c32("bout")[:, 0:1])
                blk[j]["dh2"] = d2

            def t6():
                pn2 = tailp.tile([128, 512], F32, tag="tps")
                nc.tensor.transpose(pn2[:, 0:128], blk[j]["dh2"][:],
                                    c32("identf"))
                x2 = tpool.tile([128, 128], F32, tag="x2")
                nc.vector.tensor_tensor(x2[:], pn2[:, 0:128],
                                        blk[j]["h"][:], ALU.add)
                normalize_blk(x2[:], "ln2_%d" % j)
                # out = xh2*(maskV) * g2 + maskV*b2
                nc.vector.tensor_scalar_mul(x2[:], x2[:], maskv[:, j:j + 1])
                y = tpool.tile([128, 128], F32, tag="yj")
                nc.vector.tensor_tensor(y[:], x2[:], c32("g2rep"), ALU.mult)
                nc.vector.tensor_tensor(y[:], y[:], mvb2[:, js], ALU.add)
                nc.sync.dma_start(
                    io["out"][:].rearrange("(b p) f -> p b f", p=128)
                    [:, j, :], y[:])

            return [t1, t2, t3, t4, t5, t6]

        # ---- main loop over the h_E stream ----
        with tc.tile_pool(name="p1", bufs=2, space="PSUM") as p1, \
                tc.tile_pool(name="p2", bufs=1, space="PSUM") as p2, \
                tc.tile_pool(name="tp", bufs=2, space="PSUM") as tailp:
            for g in range(ngrp):
                he = he_q.pop(0)
                if g + 2 < ngrp:
                    he_q.append(load_group(g + 2))

                for q in range(DMA_GROUP // 2):
                    t0 = g * DMA_GROUP + 2 * q
                    # pair of TT-tiles; halves at 512-col (bank) offsets
                    ps1 = p1.tile([128, 1024], F32, tag="ps1")
                    # chunk-major: consecutive matmuls share the stationary
                    for hf in range(2):
                        t = t0 + hf
                        o = 512 * hf
                        hv_rep = hv1b[:, t * NPT:(t + 1) * NPT].unsqueeze(2) \
                            .broadcast_to([128, NPT, K])
                        nc.tensor.matmul(ps1[:, o:o + TT], cb("identb"),
                                         hv_rep, start=True, stop=False)
                    for c in range(NCHUNK):
                        for hf in range(2):
                            s = 2 * q + hf
                            o = 512 * hf
                            nc.tensor.matmul(
                                ps1[:, o:o + TT],
                                cb("w1et")[:, c * 128:(c + 1) * 128],
                                he[:, c * G_ROWS + s * TT:
                                   c * G_ROWS + (s + 1) * TT],
                                start=False, stop=(c == NCHUNK - 1))
                    g1 = wpool.tile([128, PAIR], BF16, tag="g1")
                    ps1v = ps1[:].rearrange("p (hh c) -> p hh c", hh=2)
                    nc.scalar.activation(g1[:], ps1v[:, :, 0:TT], GELU)

                    ps2 = p2.tile([128, 1024], F32, tag="ps2")
                    for hf in range(2):
                        o = 512 * hf
                        nc.tensor.matmul(ps2[:, o:o + TT], cb("w2t"),
                                         g1[:, hf * TT:(hf + 1) * TT],
                                         start=True, stop=True)
                    h2 = wpool.tile([128, PAIR], BF16, tag="h2")
                    ps2v = ps2[:].rearrange("p (hh c) -> p hh c", hh=2)
                    nc.scalar.activation(h2[:], ps2v[:, :, 0:TT], GELU,
                                         bias=c32("b2"))

                    with nc.allow_low_precision(
                            reason="k-sum accumulates in fp32; only the "
                                   "output is rounded to bf16"):
                        nc.vector.tensor_reduce(
                            m2[:, t0 * NPT:(t0 + 2) * NPT],
                            h2[:].rearrange("p (n k) -> p n k", k=K),
                            AX.X, ALU.add)

                    if tail_tasks:
                        tail_tasks.pop(0)()

                if g % 4 == 3:
                    tail_tasks.extend(make_tasks(g // 4, tailp))

            while tail_tasks:
                tail_tasks.pop(0)()


def build_nc(npc):
    rows = npc * K
    nblk = npc // 128
    nc = bacc.Bacc()
    io = {}

    def inp(name, shape, dt=F32):
        io[name] = nc.dram_tensor(name, shape, dt, kind="ExternalInput")

    inp("h_et", [NCHUNK, 128, rows], BF16)
    inp("hv_t", [128, npc], BF16)
    inp("hv_nat", [npc, H])
    inp("s_mask", [128, nblk])
    inp("maskv_nat", [128, nblk])
    inp("pk32", [128, PK32_COLS])
    inp("pkb", [128, PKB_COLS], BF16)
    io["out"] = nc.dram_tensor("out", [npc, H], F32, kind="ExternalOutput")
    _emit(nc, io, npc)
    return nc


def _gelu(x):
    try:
        from scipy.special import erf
        return 0.5 * x * (1.0 + erf(x / np.sqrt(2.0)))
    except Exception:
        import math
        v = np.vectorize(
            lambda t: 0.5 * t * (1.0 + math.erf(t / 1.4142135623730951)))
        return v(x).astype(np.float64)


def prep_maps(h_V, h_E, mask_V, mask_attend,
              W1_w, W1_b, W2_w, W2_b, W3_w, W3_b,
              ln1_g, ln1_b, ln2_g, ln2_b,
              Win_w, Win_b, Wout_w, Wout_b, ncores):
    import ml_dtypes
    f32 = np.float32
    bf16 = ml_dtypes.bfloat16
    B, N, Kk, _ = h_E.shape
    nodes = B * N
    npc = nodes // ncores
    rows = npc * Kk
    nblk = npc // 128

    W1 = np.asarray(W1_w, f32)
    W1E = W1[:, H:]

    hE = np.asarray(h_E, f32)
    mA = np.asarray(mask_attend, f32)
    if not np.all(mA == 1.0):
        # replace masked neighbor features by v with W1E @ v = -POISON:
        # gelu output becomes exactly 0 there; the constant gelu(b2) that
        # then leaks into the K-sum is corrected through the W3 constants.
        v = np.linalg.lstsq(W1E.astype(np.float64),
                            np.full(H, -POISON, np.float64), rcond=None)[0]
        hE = np.where(mA[..., None] == 0.0, v.astype(f32), hE)

    hE = hE.reshape(ncores, npc, Kk, NIN)
    h_et = np.ascontiguousarray(hE.transpose(0, 3, 1, 2)).reshape(
        ncores, NCHUNK, 128, rows).astype(bf16)
    hv = np.asarray(h_V, f32).reshape(ncores, npc, H)
    hv_t = np.ascontiguousarray(hv.transpose(0, 2, 1)).astype(bf16)
    s_mask_h = mA.reshape(ncores, nblk, 128, Kk).sum(axis=3)  # [c, b, 128]
    s_mask_h = np.ascontiguousarray(s_mask_h.transpose(0, 2, 1))  # [c,128,b]
    mV = np.asarray(mask_V, f32).reshape(ncores, nblk, 128)
    maskv_nat = np.ascontiguousarray(mV.transpose(0, 2, 1))

    def t(x):
        return np.asarray(x, f32).T

    rep = lambda v: np.tile(np.asarray(v, f32).reshape(1, -1), (128, 1))

    # mask correction constants
    gelu_b2 = _gelu(np.asarray(W2_b, np.float64))
    w3gb2 = (np.asarray(W3_w, np.float64) / SCALE) @ gelu_b2  # [128]
    b3c = np.asarray(W3_b, np.float64) / SCALE + w3gb2        # per-node coeff

    pk32 = np.zeros((128, PK32_COLS), f32)

    def put32(name, arr):
        o, w = _PK32[name]
        pk32[:, o:o + w] = arr

    put32("b1", np.asarray(W1_b, f32).reshape(128, 1))
    put32("b2", np.asarray(W2_b, f32).reshape(128, 1))
    put32("b3rep", rep(b3c.astype(f32)))
    binp_f = (np.asarray(Win_b, np.float64)
              + np.asarray(Win_w, np.float64) @ np.asarray(ln1_b, np.float64))
    put32("binp", binp_f.astype(f32).reshape(FCHUNK, 128).T)
    put32("bout", np.asarray(Wout_b, f32).reshape(128, 1))
    put32("g1rep", rep(ln1_g))
    put32("b1rep", rep(ln1_b))
    put32("g2rep", rep(ln2_g))
    put32("b2rep", rep(ln2_b))
    put32("identf", np.eye(128, dtype=f32))
    put32("epsv", np.full((128, 1), EPS, f32))
    put32("dhbias", (-Kk * w3gb2).astype(f32).reshape(128, 1))
    magic = np.frombuffer(
        np.full(4, 0x5f3759df, np.uint32).tobytes(), dtype=f32)
    put32("magic4", np.tile(magic.reshape(1, 4), (128, 1))[:, :4])

    pkb = np.zeros((128, PKB_COLS), f32)

    def putb(name, arr):
        o, w = _PKB[name]
        pkb[:, o:o + w] = arr

    putb("w1et", W1E.T.reshape(NCHUNK, 128, 128).transpose(1, 0, 2)
         .reshape(128, 384))
    putb("w1vt", t(W1[:, :H]))
    putb("w2t", t(W2_w))
    putb("w3t", t(np.asarray(W3_w, f32) / SCALE))
    putb("wint", t(Win_w) * np.asarray(ln1_g, f32).reshape(128, 1))
    putb("woutt", np.asarray(Wout_w, f32).T.reshape(
        FCHUNK, 128, 128).transpose(1, 0, 2).reshape(128, 512))
    putb("identb", np.eye(128, dtype=f32))

    shared = {
        "pk32": pk32,
        "pkb": pkb.astype(bf16),
    }
    in_maps = []
    for c in range(ncores):
        m = dict(shared)
        m["h_et"] = h_et[c]
        m["hv_t"] = hv_t[c]
        m["hv_nat"] = np.ascontiguousarray(hv[c])
        m["s_mask"] = s_mask_h[c].astype(f32)
        m["maskv_nat"] = maskv_nat[c]
        in_maps.append(m)
    return in_maps, npc


_NC_CACHE = {}


def _get_nc(npc):
    if npc not in _NC_CACHE:
        nc = build_nc(npc)
        nc.finalize()
        _NC_CACHE[npc] = nc
    return _NC_CACHE[npc]


def run(inputs, trace=False):
    B, N, _, _ = inputs["h_E"].shape
    in_maps, npc = prep_maps(ncores=NCORES, **inputs)
    nc = _get_nc(npc)
    res = run_bass_kernel_spmd(nc, in_maps, core_ids=list(range(NCORES)),
                               trace=trace)
    out = np.concatenate([res.results[c]["out"] for c in range(NCORES)],
                         axis=0).reshape(B, N, H).astype(np.float32)
    return out, res.exec_time_ns


def kernel(**inputs) -> np.ndarray:
    out, _ = run(inputs)
    return out


# revision 20
# speedup vs baseline: 1.0444x; 1.0444x over previous
"""Trainium2 Bass kernel for nn_DecoderLayer (gnn_message_passing).

Sharding: flatten B*N = 4096 nodes, 512 nodes per core across 8 cores.
Device layout is feature-on-partition (transposed); h_E is pre-transposed and
cast to bf16 on host so the big stream DMAs straight into bf16 matmuls.

Math per node n, neighbor k (reference):
  h_EV = [h_V[n], h_E[n,k]]                                (128+384)
  h1 = gelu(h_EV @ W1.T + b1); h2 = gelu(h1 @ W2.T + b2)
  msg = h2 @ W3.T + b3
  dh  = sum_k mask_attend[n,k] * msg / 30
  h   = LN1(h_V + dh)
  dh2 = gelu(h @ Win.T + bin) @ Wout.T + bout
  out = mask_V[n] * LN2(h + dh2)

Structure (from trace analysis):
  - the scalar (ACT) engine is the hard floor: two full gelu passes over
    B*N*K rows at 1 elem/lane/cycle.  Everything else hides behind it.
  - stream path all bf16 (fast PE mode + FWL; fp32 matmuls run a slow mode
    and draw the HAM power throttle).
  - W1 split: hv1 = W1V^T h_V + b1 computed once per node, broadcast across
    K into PSUM via a bf16 identity matmul with a stride-0 moving AP.
  - mask_attend folded into the data host-side (poisoned h_E rows make the
    masked gelu exactly 0; the constant gelu(b2) leak is corrected through
    the W3 constants).  No mask work on the device.
  - K-sum before W3 (linear commutes): m2[n] = sum_k h2.
  - the per-node tail (W3, LN1, FFN, LN2, store) is cut into 6 tasks per
    128-node block and drip-fed into the stream one task per matmul pair,
    so it overlaps the ACT-bound stream instead of serializing at the end.
  - LayerNorm entirely on the vector engine (rsqrt via bitcast-Newton):
    the ACT table never leaves the gelu set, so no 2.7us table swaps.
"""

from contextlib import ExitStack

import numpy as np

import concourse.bacc as bacc
import concourse.tile as tile
from concourse import mybir
from concourse.bass_utils import run_bass_kernel_spmd

F32 = mybir.dt.float32
BF16 = mybir.dt.bfloat16
I32 = mybir.dt.int32
AF = mybir.ActivationFunctionType
ALU = mybir.AluOpType
AX = mybir.AxisListType

H = 128
NIN = 384
FF = 4 * H
NCHUNK = NIN // 128  # 3
FCHUNK = FF // 128   # 4
K = 48
SCALE = 30.0
EPS = 1e-5
NCORES = 8
POISON = 50.0

TT = 384                 # rows per matmul tile (8 nodes * 48)
NPT = TT // K            # 8 nodes per tile
PAIR = 2 * TT            # rows per activation/DVE pass
DMA_GROUP = 4            # tiles per DMA load
G_ROWS = TT * DMA_GROUP  # 1536 rows (1.18 MB bf16) per load

GELU = AF.Gelu

# packed f32 const layout (columns)
_PK32 = {"b1": (0, 1), "b2": (1, 1), "epsv": (2, 1), "dhbias": (3, 1),
         "magic4": (4, 4), "bout": (8, 1), "binp": (9, 4),
         "b3rep": (16, 128), "g1rep": (144, 128), "b1rep": (272, 128),
         "g2rep": (400, 128), "b2rep": (528, 128), "identf": (656, 128)}
PK32_HOT = 16
PK32_COLS = 784
# packed bf16 const layout
_PKB = {"w1vt": (0, 128), "identb": (128, 128), "w1et": (256, NCHUNK * 128),
        "w2t": (640, 128), "w3t": (768, 128), "wint": (896, FF),
        "woutt": (1408, FCHUNK * 128)}
PKB_HOT = 768
PKB_COLS = 1920


def _emit(nc, io, npc):
    rows = npc * K
    ngrp = rows // G_ROWS
    nblk = npc // 128
    assert rows % G_ROWS == 0 and npc % 128 == 0

    with tile.TileContext(nc) as tc, ExitStack() as ctx:
        cpool = ctx.enter_context(tc.tile_pool(name="const", bufs=1))
        small = ctx.enter_context(tc.tile_pool(name="small", bufs=8))
        hpool = ctx.enter_context(tc.tile_pool(name="he", bufs=4))
        wpool = ctx.enter_context(tc.tile_pool(name="work", bufs=6))
        tpool = ctx.enter_context(tc.tile_pool(name="tail", bufs=2))

        # ---- packed constants (few big DMAs) ----
        # warm the Gelu LUT first; input from a memset tile so the table
        # load doesn't wait on any DMA
        wz = small.tile([128, 1], F32, tag="wz")
        nc.gpsimd.memset(wz[:], 0.0)
        warm = small.tile([128, 1], F32, tag="warm")
        nc.scalar.activation(warm[:], wz[:], GELU)

        # hot weights ride the fast sync DMA ring (~3.5x the gpsimd
        # ring's effective bandwidth) so the stream starts sooner
        pkb = cpool.tile([128, PKB_COLS], BF16, tag="pkb")
        nc.sync.dma_start(pkb[:, 0:PKB_HOT], io["pkb"][:, 0:PKB_HOT])
        pk32 = cpool.tile([128, PK32_COLS], F32, tag="pk32")
        nc.gpsimd.dma_start(pk32[:, 0:PK32_HOT], io["pk32"][:, 0:PK32_HOT])
        hv_tb = cpool.tile([128, npc], BF16, tag="hv_tb")
        nc.gpsimd.dma_start(hv_tb[:], io["hv_t"][:])
        nc.gpsimd.dma_start(pkb[:, PKB_HOT:], io["pkb"][:, PKB_HOT:])
        nc.gpsimd.dma_start(pk32[:, PK32_HOT:], io["pk32"][:, PK32_HOT:])

        def c32(name):
            o, w = _PK32[name]
            return pk32[:, o:o + w]

        def cb(name):
            o, w = _PKB[name]
            return pkb[:, o:o + w]

        hv_nat = cpool.tile([128, nblk * 128], F32, tag="hv_nat")
        nc.gpsimd.dma_start(
            hv_nat[:], io["hv_nat"][:].rearrange("(b p) f -> p b f", p=128))
        s_mask = cpool.tile([128, nblk], F32, tag="s_mask")
        nc.gpsimd.dma_start(s_mask[:], io["s_mask"][:])
        maskv = cpool.tile([128, nblk], F32, tag="maskv")
        nc.gpsimd.dma_start(maskv[:], io["maskv_nat"][:])

        h_et = io["h_et"][:]            # [NCHUNK, 128, rows] bf16

        def load_group(g):
            he = hpool.tile([128, NCHUNK * G_ROWS], BF16, tag="he")
            r0 = g * G_ROWS
            # src (p, c, r) enumeration to match dest free layout (c, r)
            nc.sync.dma_start(
                he[:], h_et[:, :, r0:r0 + G_ROWS].transpose([1, 0, 2]))
            return he

        he_q = [load_group(0), load_group(1), load_group(2)]  # prefetch

        m2 = cpool.tile([128, npc], BF16, tag="m2")

        # hv1 = W1V^T @ h_V + b1, computed once, rounded to bf16
        hv1b = cpool.tile([128, npc], BF16, tag="hv1b")
        with tc.tile_pool(name="pp0", bufs=1, space="PSUM") as pp0:
            ps_hv = pp0.tile([128, npc], F32, tag="pp0")
            nc.tensor.matmul(ps_hv[:], cb("w1vt"), hv_tb[:],
                             start=True, stop=True)
            nc.scalar.activation(hv1b[:], ps_hv[:], AF.Identity,
                                 bias=c32("b1"))

        # per-node constant terms, precomputed off the critical path:
        # hvterm = h_V + s_mask*b3c (enters LN1 residual);
        # mvb2 = mask_V * ln2_b (enters the masked output)
        hvterm = cpool.tile([128, nblk * 128], F32, tag="hvterm")
        nc.vector.tensor_tensor(
            hvterm[:].rearrange("p (b f) -> p b f", f=128),
            s_mask[:].unsqueeze(2).broadcast_to([128, nblk, 128]),
            c32("b3rep").unsqueeze(1).broadcast_to([128, nblk, 128]),
            ALU.mult)
        nc.vector.tensor_tensor(hvterm[:], hvterm[:], hv_nat[:], ALU.add)
        mvb2 = cpool.tile([128, nblk * 128], F32, tag="mvb2")
        nc.vector.tensor_tensor(
            mvb2[:].rearrange("p (b f) -> p b f", f=128),
            maskv[:].unsqueeze(2).broadcast_to([128, nblk, 128]),
            c32("b2rep").unsqueeze(1).broadcast_to([128, nblk, 128]),
            ALU.mult)

        def rsqrt_blk(v1, pfx):
            # rstd = 1/sqrt(v1) on [128,1] via bitcast + 2 Newton steps
            yi = small.tile([128, 1], I32, tag=pfx + "yi")
            nc.vector.tensor_scalar(yi[:], v1.bitcast(I32), 1, None,
                                    ALU.arith_shift_right)
            nc.vector.tensor_tensor(yi[:], c32("magic4")[:, 0:1].bitcast(I32),
                                    yi[:], ALU.subtract)
            y = yi[:].bitcast(F32)
            s = small.tile([128, 1], F32, tag=pfx + "s")
            for _ in range(1):  # y *= 1.5 - 0.5*v*y*y
                nc.vector.tensor_tensor(s[:], y, y, ALU.mult)
                nc.vector.tensor_tensor(s[:], s[:], v1, ALU.mult)
                nc.vector.tensor_scalar(s[:], s[:], -0.5, 1.5,
                                        ALU.mult, ALU.add)
                nc.vector.tensor_tensor(y, y, s[:], ALU.mult)
            return yi[:].bitcast(F32)

        def normalize_blk(x, pfx):
            # x: [128,128] fp32, normalized in place to (x-mu)/std
            mu = small.tile([128, 1], F32, tag=pfx + "mu")
            nc.vector.tensor_reduce(mu[:], x, AX.X, ALU.add)
            nc.vector.tensor_scalar_mul(mu[:], mu[:], 1.0 / 128.0)
            nc.vector.tensor_scalar_sub(x, x, mu[:, 0:1])
            sq = tpool.tile([128, 128], F32, tag="lnsq")
            nc.vector.tensor_tensor(sq[:], x, x, ALU.mult)
            v1 = small.tile([128, 1], F32, tag=pfx + "v1")
            nc.vector.tensor_reduce(v1[:], sq[:], AX.X, ALU.add)
            nc.vector.tensor_scalar(v1[:], v1[:], 1.0 / 128.0, EPS,
                                    ALU.mult, ALU.add)
            rstd = rsqrt_blk(v1[:], pfx)
            nc.vector.tensor_scalar_mul(x, x, rstd[:, 0:1])

        # ---- per-block tail tasks (drip-fed into the stream) ----
        blk = [{} for _ in range(nblk)]
        tail_tasks = []

        def make_tasks(j, tailp):
            js = slice(j * 128, (j + 1) * 128)

            def t1():
                ps = tailp.tile([128, 512], F32, tag="tps")
                nc.tensor.matmul(ps[:, 0:128], cb("w3t"), m2[:, js],
                                 start=True, stop=True)
                d = tpool.tile([128, 128], F32, tag="dht")
                nc.vector.tensor_scalar_add(d[:], ps[:, 0:128],
                                            c32("dhbias")[:, 0:1])
                blk[j]["dht"] = d

            def t2():
                pn = tailp.tile([128, 512], F32, tag="tps")
                nc.tensor.transpose(pn[:, 0:128], blk[j]["dht"][:],
                                    c32("identf"))
                x = tpool.tile([128, 128], F32, tag="x1")
                nc.vector.tensor_tensor(x[:], pn[:, 0:128], hvterm[:, js],
                                        ALU.add)
                normalize_blk(x[:], "ln1_%d" % j)
                blk[j]["xh"] = x

            def t3():
                # FFN path: ln1 affine is folded into wint/binp on host, so
                # the transpose consumes the bare normalized x-hat
                pt = tailp.tile([128, 512], F32, tag="tps")
                nc.tensor.transpose(pt[:, 0:128], blk[j]["xh"][:],
                                    c32("identf"))
                ht = tpool.tile([128, 128], BF16, tag="ht")
                nc.vector.tensor_scalar_add(ht[:], pt[:, 0:128], 0.0)
                blk[j]["ht"] = ht
                # residual-path h = xh*g1 + b1, off the FFN critical chain
                h = tpool.tile([128, 128], F32, tag="hj")
                nc.vector.tensor_tensor(h[:], blk[j]["xh"][:], c32("g1rep"),
                                        ALU.mult)
                nc.vector.tensor_tensor(h[:], h[:], c32("b1rep"), ALU.add)
                blk[j]["h"] = h

            def t4():
                pf = tailp.tile([128, 512], F32, tag="tps")
                for jo in range(FCHUNK):
                    nc.tensor.matmul(
                        pf[:, jo * 128:(jo + 1) * 128],
                        cb("wint")[:, jo * 128:(jo + 1) * 128],
                        blk[j]["ht"][:], start=True, stop=True)
                fz = tpool.tile([128, 512], BF16, tag="fz")
                nc.vector.tensor_tensor(
                    fz[:].rearrange("p (c f) -> p c f", f=128),
                    pf[:].rearrange("p (c f) -> p c f", f=128),
                    c32("binp").unsqueeze(2).broadcast_to([128, FCHUNK, 128]),
                    ALU.add)
                fb = tpool.tile([128, 512], BF16, tag="fb")
                nc.scalar.activation(fb[:], fz[:], GELU)
                blk[j]["fb"] = fb

            def t5():
                ps2_ = tailp.tile([128, 512], F32, tag="tps")
                for jf in range(FCHUNK):
                    nc.tensor.matmul(
                        ps2_[:, 0:128],
                        cb("woutt")[:, jf * 128:(jf + 1) * 128],
                        blk[j]["fb"][:, jf * 128:(jf + 1) * 128],
                        start=(jf == 0), stop=(jf == FCHUNK - 1))
                d2 = tpool.tile([128, 128], F32, tag="dh2")
                nc.vector.tensor_scalar_add(d2[:], ps2_[:, 0:128],
                                            c32("bout")[:, 0:1])
                blk[j]["dh2"] = d2

            def t6():
                pn2 = tailp.tile([128, 512], F32, tag="tps")
                nc.tensor.transpose(pn2[:, 0:128], blk[j]["dh2"][:],
                                    c32("identf"))
                x2 = tpool.tile([128, 128], F32, tag="x2")
                nc.vector.tensor_tensor(x2[:], pn2[:, 0:128],
                                        blk[j]["h"][:], ALU.add)
                normalize_blk(x2[:], "ln2_%d" % j)
                # out = xh2*(maskV) * g2 + maskV*b2
                nc.vector.tensor_scalar_mul(x2[:], x2[:], maskv[:, j:j + 1])
                y = tpool.tile([128, 128], F32, tag="yj")
                nc.vector.tensor_tensor(y[:], x2[:], c32("g2rep"), ALU.mult)
                nc.vector.tensor_tensor(y[:], y[:], mvb2[:, js], ALU.add)
                nc.sync.dma_start(
                    io["out"][:].rearrange("(b p) f -> p b f", p=128)
                    [:, j, :], y[:])

            return [t1, t2, t3, t4, t5, t6]

        # ---- main loop over the h_E stream ----
        with tc.tile_pool(name="p1", bufs=2, space="PSUM") as p1, \
                tc.tile_pool(name="p2", bufs=1, space="PSUM") as p2, \
                tc.tile_pool(name="tp", bufs=2, space="PSUM") as tailp:
            for g in range(ngrp):
                he = he_q.pop(0)
                if g + 3 < ngrp:
                    he_q.append(load_group(g + 3))

                for q in range(DMA_GROUP // 2):
                    t0 = g * DMA_GROUP + 2 * q
                    # pair of TT-tiles; halves at 512-col (bank) offsets
                    ps1 = p1.tile([128, 1024], F32, tag="ps1")
                    # chunk-major: consecutive matmuls share the stationary
                    for hf in range(2):
                        t = t0 + hf
                        o = 512 * hf
                        hv_rep = hv1b[:, t * NPT:(t + 1) * NPT].unsqueeze(2) \
                            .broadcast_to([128, NPT, K])
                        nc.tensor.matmul(ps1[:, o:o + TT], cb("identb"),
                                         hv_rep, start=True, stop=False)
                    for c in range(NCHUNK):
                        for hf in range(2):
                            s = 2 * q + hf
                            o = 512 * hf
                            nc.tensor.matmul(
                                ps1[:, o:o + TT],
                                cb("w1et")[:, c * 128:(c + 1) * 128],
                                he[:, c * G_ROWS + s * TT:
                                   c * G_ROWS + (s + 1) * TT],
                                start=False, stop=(c == NCHUNK - 1))
                    g1 = wpool.tile([128, PAIR], BF16, tag="g1")
                    ps1v = ps1[:].rearrange("p (hh c) -> p hh c", hh=2)
                    nc.scalar.activation(g1[:], ps1v[:, :, 0:TT], GELU)

                    ps2 = p2.tile([128, 1024], F32, tag="ps2")
                    for hf in range(2):
                        o = 512 * hf
                        nc.tensor.matmul(ps2[:, o:o + TT], cb("w2t"),
                                         g1[:, hf * TT:(hf + 1) * TT],
                                         start=True, stop=True)
                    h2 = wpool.tile([128, PAIR], BF16, tag="h2")
                    ps2v = ps2[:].rearrange("p (hh c) -> p hh c", hh=2)
                    nc.scalar.activation(h2[:], ps2v[:, :, 0:TT], GELU,
                                         bias=c32("b2"))

                    with nc.allow_low_precision(
                            reason="k-sum accumulates in fp32; only the "
                                   "output is rounded to bf16"):
                        nc.vector.tensor_reduce(
                            m2[:, t0 * NPT:(t0 + 2) * NPT],
                            h2[:].rearrange("p (n k) -> p n k", k=K),
                            AX.X, ALU.add)

                    if tail_tasks:
                        tail_tasks.pop(0)()

                if g % 4 == 3:
                    tail_tasks.extend(make_tasks(g // 4, tailp))

            while tail_tasks:
                tail_tasks.pop(0)()


def build_nc(npc):
    rows = npc * K
    nblk = npc // 128
    nc = bacc.Bacc()
    io = {}

    def inp(name, shape, dt=F32):
        io[name] = nc.dram_tensor(name, shape, dt, kind="ExternalInput")

    inp("h_et", [NCHUNK, 128, rows], BF16)
    inp("hv_t", [128, npc], BF16)
    inp("hv_nat", [npc, H])
    inp("s_mask", [128, nblk])
    inp("maskv_nat", [128, nblk])
    inp("pk32", [128, PK32_COLS])
    inp("pkb", [128, PKB_COLS], BF16)
    io["out"] = nc.dram_tensor("out", [npc, H], F32, kind="ExternalOutput")
    _emit(nc, io, npc)
    return nc


def _gelu(x):
    try:
        from scipy.special import erf
        return 0.5 * x * (1.0 + erf(x / np.sqrt(2.0)))
    except Exception:
        import math
        v = np.vectorize(
            lambda t: 0.5 * t * (1.0 + math.erf(t / 1.4142135623730951)))
        return v(x).astype(np.float64)


def prep_maps(h_V, h_E, mask_V, mask_attend,
              W1_w, W1_b, W2_w, W2_b, W3_w, W3_b,
              ln1_g, ln1_b, ln2_g, ln2_b,
              Win_w, Win_b, Wout_w, Wout_b, ncores):
    import ml_dtypes
    f32 = np.float32
    bf16 = ml_dtypes.bfloat16
    B, N, Kk, _ = h_E.shape
    nodes = B * N
    npc = nodes // ncores
    rows = npc * Kk
    nblk = npc // 128

    W1 = np.asarray(W1_w, f32)
    W1E = W1[:, H:]

    hE = np.asarray(h_E, f32)
    mA = np.asarray(mask_attend, f32)
    if not np.all(mA == 1.0):
        # replace masked neighbor features by v with W1E @ v = -POISON:
        # gelu output becomes exactly 0 there; the constant gelu(b2) that
        # then leaks into the K-sum is corrected through the W3 constants.
        v = np.linalg.lstsq(W1E.astype(np.float64),
                            np.full(H, -POISON, np.float64), rcond=None)[0]
        hE = np.where(mA[..., None] == 0.0, v.astype(f32), hE)

    hE = hE.reshape(ncores, npc, Kk, NIN)
    h_et = np.ascontiguousarray(hE.transpose(0, 3, 1, 2)).reshape(
        ncores, NCHUNK, 128, rows).astype(bf16)
    hv = np.asarray(h_V, f32).reshape(ncores, npc, H)
    hv_t = np.ascontiguousarray(hv.transpose(0, 2, 1)).astype(bf16)
    s_mask_h = mA.reshape(ncores, nblk, 128, Kk).sum(axis=3)  # [c, b, 128]
    s_mask_h = np.ascontiguousarray(s_mask_h.transpose(0, 2, 1))  # [c,128,b]
    mV = np.asarray(mask_V, f32).reshape(ncores, nblk, 128)
    maskv_nat = np.ascontiguousarray(mV.transpose(0, 2, 1))

    def t(x):
        return np.asarray(x, f32).T

    rep = lambda v: np.tile(np.asarray(v, f32).reshape(1, -1), (128, 1))

    # mask correction constants
    gelu_b2 = _gelu(np.asarray(W2_b, np.float64))
    w3gb2 = (np.asarray(W3_w, np.float64) / SCALE) @ gelu_b2  # [128]
    b3c = np.asarray(W3_b, np.float64) / SCALE + w3gb2        # per-node coeff

    pk32 = np.zeros((128, PK32_COLS), f32)

    def put32(name, arr):
        o, w = _PK32[name]
        pk32[:, o:o + w] = arr

    put32("b1", np.asarray(W1_b, f32).reshape(128, 1))
    put32("b2", np.asarray(W2_b, f32).reshape(128, 1))
    put32("b3rep", rep(b3c.astype(f32)))
    binp_f = (np.asarray(Win_b, np.float64)
              + np.asarray(Win_w, np.float64) @ np.asarray(ln1_b, np.float64))
    put32("binp", binp_f.astype(f32).reshape(FCHUNK, 128).T)
    put32("bout", np.asarray(Wout_b, f32).reshape(128, 1))
    put32("g1rep", rep(ln1_g))
    put32("b1rep", rep(ln1_b))
    put32("g2rep", rep(ln2_g))
    put32("b2rep", rep(ln2_b))
    put32("identf", np.eye(128, dtype=f32))
    put32("epsv", np.full((128, 1), EPS, f32))
    put32("dhbias", (-Kk * w3gb2).astype(f32).reshape(128, 1))
    magic = np.frombuffer(
        np.full(4, 0x5f3759df, np.uint32).tobytes(), dtype=f32)
    put32("magic4", np.tile(magic.reshape(1, 4), (128, 1))[:, :4])

    pkb = np.zeros((128, PKB_COLS), f32)

    def putb(name, arr):
        o, w = _PKB[name]
        pkb[:, o:o + w] = arr

    putb("w1et", W1E.T.reshape(NCHUNK, 128, 128).transpose(1, 0, 2)
         .reshape(128, 384))
    putb("w1vt", t(W1[:, :H]))
    putb("w2t", t(W2_w))
    putb("w3t", t(np.asarray(W3_w, f32) / SCALE))
    putb("wint", t(Win_w) * np.asarray(ln1_g, f32).reshape(128, 1))
    putb("woutt", np.asarray(Wout_w, f32).T.reshape(
        FCHUNK, 128, 128).transpose(1, 0, 2).reshape(128, 512))
    putb("identb", np.eye(128, dtype=f32))

    shared = {
        "pk32": pk32,
        "pkb": pkb.astype(bf16),
    }
    in_maps = []
    for c in range(ncores):
        m = dict(shared)
        m["h_et"] = h_et[c]
        m["hv_t"] = hv_t[c]
        m["hv_nat"] = np.ascontiguousarray(hv[c])
        m["s_mask"] = s_mask_h[c].astype(f32)
        m["maskv_nat"] = maskv_nat[c]
        in_maps.append(m)
    return in_maps, npc


_NC_CACHE = {}


def _get_nc(npc):
    if npc not in _NC_CACHE:
        nc = build_nc(npc)
        nc.finalize()
        _NC_CACHE[npc] = nc
    return _NC_CACHE[npc]


def run(inputs, trace=False):
    B, N, _, _ = inputs["h_E"].shape
    in_maps, npc = prep_maps(ncores=NCORES, **inputs)
    nc = _get_nc(npc)
    res = run_bass_kernel_spmd(nc, in_maps, core_ids=list(range(NCORES)),
                               trace=trace)
    out = np.concatenate([res.results[c]["out"] for c in range(NCORES)],
                         axis=0).reshape(B, N, H).astype(np.float32)
    return out, res.exec_time_ns


def kernel(**inputs) -> np.ndarray:
    out, _ = run(inputs)
    return out
